# revision 5
# baseline (speedup 1.0000x reference)
import sys
for _p in ("/opt/trn_rl_repo",):
    if _p not in sys.path:
        sys.path.insert(0, _p)
"""GAT 2-layer kernel for TRN2, 8-core dst-sharded — v7 "split collectives, contiguous halves".

Gather cost on HW is ~8 ns per descriptor regardless of payload size, so
v3 issues ONE descriptor per edge (vs v2's two parity passes): the int16
index addresses a PAIR of adjacent table rows (25k pairs < 32767), the
descriptor fetches both rows (1280 B), and a per-slot 0/1 mask selects
the right parity on the Vector engine (2 multiplies + 1 add, bf16-exact).

Other changes vs v2:
  * residual/table matmuls emit node-major rows directly (lhs=x-window),
    eliminating the per-batch DMA transposes that stalled the Sync queue
  * the K-neighbor reduction is a log-tree of contiguous adds into an
    fp32 temp (the strided DVE reduce ran at ~0.5 elem/ns)
  * gathers are chunk-pipelined into a small staging tile and selected
    immediately, so batch i+1's gathers overlap batch i's vector work
"""

import numpy as np
import ml_dtypes
from contextlib import ExitStack

import concourse.bass as bass
import concourse.bacc as bacc
import concourse.mybir as mybir
import concourse.tile as tile

dt = mybir.dt
F32 = dt.float32
BF16 = dt.bfloat16
AL = mybir.AluOpType
ACT = mybir.ActivationFunctionType
BF = ml_dtypes.bfloat16

NEG_SLOPE = 0.2
_CORES = 8
WIN = 128
GCAP = 896
GPC = 2        # gather chunks per select group
MAX_TILES = 44
NQ = 4          # swdge queues for gather chunks


def wrap_idx(iv):
    """[n*16] int -> [128, n] int16 gather layout (t -> (t%16, t//16))."""
    iv = np.asarray(iv, np.int64)
    assert len(iv) % 16 == 0
    cols = len(iv) // 16
    a = iv.reshape(cols, 16).T.astype(np.int16)
    return np.tile(a, (8, 1))


def slot_grid(v, fill):
    """[TOTSLOT] values -> [128, TOTSLOT//128] (slot s -> [s%128, s//128])."""
    assert len(v) % 128 == 0
    return np.ascontiguousarray(np.asarray(v).reshape(-1, 128).T.astype(fill))


def make_cfg(N, E, F_in, H, D, OUT, cores):
    own = (N + cores - 1) // cores
    nw = (own + WIN - 1) // WIN
    ownp = nw * WIN
    assert ownp % 2 == 0
    cfg = dict(
        N=N, E=E, F_IN=F_in, H=H, D=D, OUT=OUT, CORES=cores, OWN=own,
        NW=nw, OWNP=ownp,
        GROW0=320, GROW1=128,          # gathered row elems per node
        C0=F_in + 2 * H,               # table0 matmul cols (h | al_s | al_d)
        C1=OUT + 2 + OUT,              # table1 cols (hW1 | als | ald | res)
        PROWS=1 + cores * ownp // 2,   # pair rows (row 0 = PAD sentinel)
        SPL=(nw // 2 + 1) * WIN,       # row split for the half collectives
    )
    assert cfg["PROWS"] - 1 <= 32767
    return cfg


def prepare(x, edge_index, weights, cfg):
    N, E, C, OWN, OWNP, NW = (cfg[k] for k in
                              ("N", "E", "CORES", "OWN", "OWNP", "NW"))
    H, D, F_IN, OUT = cfg["H"], cfg["D"], cfg["F_IN"], cfg["OUT"]

    src = np.concatenate([np.asarray(edge_index[0]),
                          np.arange(N)]).astype(np.int64)
    dst = np.concatenate([np.asarray(edge_index[1]),
                          np.arange(N)]).astype(np.int64)

    # degree-balanced relabeling: node order[i] -> (core i%C, pos i//C)
    deg = np.bincount(dst, minlength=N)
    order = np.argsort(-deg, kind="stable")
    core_of = np.empty(N, np.int64)
    pos_of = np.empty(N, np.int64)
    core_of[order] = np.arange(N) % C
    pos_of[order] = np.arange(N) // C

    # per-window max lane count (shared across cores)
    cnt = np.zeros((C, OWNP), np.int64)
    np.add.at(cnt, (core_of[dst], pos_of[dst]), 1)
    Kw = cnt.reshape(C, NW, WIN).max(axis=(0, 2))

    # batches of windows with uniform K, capped at MAX_TILES j-tiles
    batches = []
    w = 0
    while w < NW:
        K = max(int(Kw[w]), 1)
        assert K <= MAX_TILES, f"window degree {K} exceeds MAX_TILES"
        nw_b = 1
        while w + nw_b < NW:
            K2 = max(K, int(Kw[w + nw_b]))
            if (nw_b + 1) * K2 > MAX_TILES:
                break
            K = K2
            nw_b += 1
        batches.append((w, nw_b, K))
        w += nw_b

    slot0_w = np.zeros(NW, np.int64)
    base = 0
    for (w0, nw_b, K) in batches:
        for wl in range(nw_b):
            slot0_w[w0 + wl] = base + wl * K * WIN
        base += nw_b * K * WIN
    totslot = base
    assert totslot % 128 == 0

    # per-edge slot assignment: j = rank within (core,pos) group
    ec, ep = core_of[dst], pos_of[dst]
    order_e = np.lexsort((src, ep, ec))
    src_s, ec_s, ep_s = src[order_e], ec[order_e], ep[order_e]
    grp = ec_s * OWNP + ep_s
    starts = np.searchsorted(grp, np.arange(C * OWNP))
    j_of = np.arange(len(grp)) - starts[grp]
    wd = ep_s // WIN
    lane = ep_s % WIN
    slot = slot0_w[wd] + j_of * WIN + lane
    # pair layout: [1 .. 1+C*HA) = rows [0,SPL) of each core in core order,
    # then rows [SPL,OWNP) — so each half all-gather writes contiguously
    SPL = cfg["SPL"]
    HA, HB = SPL // 2, (OWNP - SPL) // 2
    sc, sp = core_of[src_s], pos_of[src_s]
    pair = np.where(sp < SPL, 1 + sc * HA + sp // 2,
                    1 + C * HA + sc * HB + (sp - SPL) // 2)
    parity = sp % 2

    in_maps = []
    for c in range(C):
        m = ec_s == c
        pr = np.zeros(totslot, np.int64)           # default: sentinel pair
        pa = np.zeros(totslot, np.int64)
        pr[slot[m]] = pair[m]
        pa[slot[m]] = parity[m]
        nodes = np.full(OWNP, -1, np.int64)
        mine = core_of == c
        nodes[pos_of[mine]] = np.where(mine)[0]
        xT = np.zeros((F_IN, OWNP), BF)
        valid = nodes >= 0
        xT[:, valid] = np.asarray(x, np.float32)[nodes[valid]].T.astype(BF)
        in_maps.append(dict(
            xT=xT,
            idxP=wrap_idx(pr),
            msk=slot_grid(pa, BF),
            mski=slot_grid(1 - pa, BF),
        ))

    # shared constants
    W0 = weights["W0"]; a_s0 = weights["a_src0"]; a_d0 = weights["a_dst0"]
    Wr0 = weights["Wr0"]; W1 = weights["W1"]; a_s1 = weights["a_src1"]
    a_d1 = weights["a_dst1"]; Wr1 = weights["Wr1"]
    blk0s = np.zeros((H * D, H), np.float32)
    blk0d = np.zeros((H * D, H), np.float32)
    for k in range(H):
        blk0s[k * D:(k + 1) * D, k] = a_s0[k]
        blk0d[k * D:(k + 1) * D, k] = a_d0[k]
    wcat0 = np.concatenate([W0, W0 @ blk0s, W0 @ blk0d], axis=1).astype(BF)
    wcat1 = np.concatenate([W1, W1 @ a_s1.reshape(-1, 1),
                            W1 @ a_d1.reshape(-1, 1), Wr1], axis=1).astype(BF)
    biasr0 = (weights["b0"] + weights["br0"]).astype(BF).reshape(1, -1)
    biasr1 = np.zeros((1, cfg["C1"]), BF)
    biasr1[0, OUT + 2:] = (weights["b1"] + weights["br1"]).astype(BF)
    consts = dict(
        wcat0=wcat0, wr0=np.asarray(Wr0, np.float32).astype(BF),
        wcat1=wcat1, biasr0=biasr0, biasr1=biasr1,
    )
    for m_ in in_maps:
        m_.update(consts)
    meta = dict(batches=batches, totslot=totslot,
                core_of=core_of, pos_of=pos_of,
                bias0_nz=bool(np.any(np.asarray(biasr0, np.float32))),
                bias1_nz=bool(np.any(np.asarray(biasr1, np.float32))))
    return in_maps, meta


def build(cfg, meta, repeat=1, abl=()):
    NOCOLL = "nocoll" in abl
    NOGATHER = "nogather" in abl
    N, C, OWN, OWNP, NW = (cfg[k] for k in
                           ("N", "CORES", "OWN", "OWNP", "NW"))
    F_IN, H, D, OUT = cfg["F_IN"], cfg["H"], cfg["D"], cfg["OUT"]
    GROW0, GROW1, C0, C1 = cfg["GROW0"], cfg["GROW1"], cfg["C0"], cfg["C1"]
    PROWS = cfg["PROWS"]
    SPLC = cfg["SPL"]
    batches = meta["batches"]
    TOTSLOT = meta["totslot"]
    BIAS0 = meta.get("bias0_nz", True)
    BIAS1 = meta.get("bias1_nz", True)
    KCH = (F_IN + 127) // 128   # 2
    AH = F_IN + H               # 264: selected row [h | al_s]
    A1 = OUT + 1                # 65: selected row [hW1 | als]

    nc = bacc.Bacc("TRN2", target_bir_lowering=False, debug=False,
                   num_devices=C, num_swdge_queues=NQ)

    _nreg_cache = {}

    def nreg(v):
        if v not in _nreg_cache:
            _nreg_cache[v] = nc.gpsimd.to_reg(v)
        return _nreg_cache[v]

    xT_in = nc.dram_tensor("xT", [F_IN, OWNP], BF16, kind="ExternalInput")
    idxP_in = nc.dram_tensor("idxP", [128, TOTSLOT // 16], dt.int16,
                             kind="ExternalInput")
    msk_in = nc.dram_tensor("msk", [128, TOTSLOT // 128], BF16,
                            kind="ExternalInput")
    mski_in = nc.dram_tensor("mski", [128, TOTSLOT // 128], BF16,
                             kind="ExternalInput")
    wcat0_in = nc.dram_tensor("wcat0", [F_IN, C0], BF16, kind="ExternalInput")
    wr0_in = nc.dram_tensor("wr0", [F_IN, F_IN], BF16, kind="ExternalInput")
    wcat1_in = nc.dram_tensor("wcat1", [F_IN, C1], BF16, kind="ExternalInput")
    biasr0_in = nc.dram_tensor("biasr0", [1, F_IN], BF16, kind="ExternalInput")
    biasr1_in = nc.dram_tensor("biasr1", [1, C1], BF16, kind="ExternalInput")
    out_own = nc.dram_tensor("out_own", [OWNP, OUT], F32, kind="ExternalOutput")

    tblg0 = nc.dram_tensor("tblg0", [OWNP, GROW0], BF16)
    tblg1 = nc.dram_tensor("tblg1", [OWNP, GROW1], BF16)
    reg0 = nc.dram_tensor("reg0", [PROWS, 2 * GROW0], BF16, addr_space="Shared")
    reg1 = nc.dram_tensor("reg1", [PROWS, 2 * GROW1], BF16, addr_space="Shared")
    rg = [list(range(C))]

    with tile.TileContext(nc) as tc, ExitStack() as ctx:
        const = ctx.enter_context(tc.tile_pool(name="const", bufs=1))
        wcat0_t = const.tile([128, KCH, C0], BF16)
        nc.sync.dma_start(wcat0_t[:], wcat0_in[:, :].rearrange(
            "(k p) c -> p k c", p=128))
        wr0_t = const.tile([128, KCH, F_IN], BF16)
        nc.sync.dma_start(wr0_t[:], wr0_in[:, :].rearrange(
            "(k p) c -> p k c", p=128))
        wcat1_t = const.tile([128, KCH, C1], BF16)
        nc.sync.dma_start(wcat1_t[:], wcat1_in[:, :].rearrange(
            "(k p) c -> p k c", p=128))
        biasr0_t = const.tile([1, F_IN], BF16)
        nc.sync.dma_start(biasr0_t[:], biasr0_in[:])
        biasr1_t = const.tile([1, C1], BF16)
        nc.sync.dma_start(biasr1_t[:], biasr1_in[:])
        ones_t = const.tile([1, 128], BF16)
        nc.vector.memset(ones_t[:], 1.0)
        idxP_t = const.tile([128, TOTSLOT // 16], dt.int16)
        nc.sync.dma_start(idxP_t[:], idxP_in[:])
        msk_t = const.tile([128, TOTSLOT // 128], BF16)
        nc.sync.dma_start(msk_t[:], msk_in[:])
        mski_t = const.tile([128, TOTSLOT // 128], BF16)
        nc.sync.dma_start(mski_t[:], mski_in[:])

        persist = ctx.enter_context(tc.tile_pool(name="persist", bufs=1))
        h1T = persist.tile([128, KCH, OWNP], BF16)
        res0L = persist.tile([128, NW, F_IN], BF16)
        ad0 = persist.tile([128, NW, H], BF16)
        ad1 = persist.tile([128, NW, 1 + OUT], BF16)

        for _rep in range(repeat):
            # ---------------- setup: table0 rows, res0, sentinels ----------
            with tc.tile_pool(name="xp", bufs=1) as xp, \
                 tc.tile_pool(name="sps", bufs=2, space="PSUM") as sps, \
                 tc.tile_pool(name="ssb", bufs=2) as ssb:
                xT_t = xp.tile([128, KCH, OWNP], BF16)
                nc.sync.dma_start(xT_t[:], xT_in[:, :].rearrange(
                    "(k p) n -> p k n", p=128))
                # split the all-gather into two halves so the first half
                # overlaps the rest of table0 + the res0 matmuls; the pair
                # index space is [half A of all cores | half B of all cores]
                # so each half-collective output is contiguous
                SPL = SPLC
                HA, HB = SPL // 2, (OWNP - SPL) // 2

                def kick0(lo, hi):
                    o0_, o1_ = ((1, 1 + C * HA) if lo == 0 else
                                (1 + C * HA, 1 + C * (HA + HB)))
                    if NOCOLL:
                        nc.sync.dma_start(
                            reg0[o0_:o0_ + (hi - lo) // 2, :],
                            tblg0[lo:hi, :].rearrange("(v t) e -> v (t e)", t=2))
                    else:
                        nc.gpsimd.collective_compute(
                            "AllGather", AL.bypass, replica_groups=rg,
                            ins=[tblg0[lo:hi, :].opt()],
                            outs=[reg0[o0_:o1_, :].opt()])

                for w in range(NW):
                    ps = sps.tile([128, C0], F32, tag="t0")
                    for k in range(KCH):
                        nc.tensor.matmul(ps[:], xT_t[:, k, w * 128:(w + 1) * 128],
                                         wcat0_t[:, k, :],
                                         start=(k == 0), stop=(k == KCH - 1))
                    st = ssb.tile([128, GROW0], BF16, tag="st0")
                    nc.scalar.copy(st[:, :AH], ps[:, :AH])
                    nc.scalar.copy(ad0[:, w, :], ps[:, AH:C0])
                    nc.sync.dma_start(tblg0[w * 128:(w + 1) * 128, :], st[:])
                    if (w + 1) * 128 == SPL:
                        kick0(0, SPL)
                kick0(SPL, OWNP)
                for w in range(NW):
                    ps = sps.tile([128, F_IN], F32, tag="r0")
                    for k in range(KCH):
                        nc.tensor.matmul(ps[:], xT_t[:, k, w * 128:(w + 1) * 128],
                                         wr0_t[:, k, :], start=(k == 0),
                                         stop=(k == KCH - 1 and not BIAS0))
                    if BIAS0:
                        nc.tensor.matmul(ps[:], ones_t[:], biasr0_t[:],
                                         start=False, stop=True)
                    nc.scalar.copy(res0L[:, w, :], ps[:])
                # PAD sentinel pair rows (al_s = -1e30 in both halves)
                s0 = ssb.tile([1, 2 * GROW0], BF16, tag="s0")
                nc.vector.memset(s0[:], 0.0)
                nc.vector.memset(s0[:, F_IN:AH], -1e30)
                nc.vector.memset(s0[:, GROW0 + F_IN:GROW0 + AH], -1e30)
                nc.sync.dma_start(reg0[0:1, :], s0[:])
                s1 = ssb.tile([1, 2 * GROW1], BF16, tag="s1")
                nc.vector.memset(s1[:], 0.0)
                nc.vector.memset(s1[:, OUT:A1], -1e30)
                nc.vector.memset(s1[:, GROW1 + OUT:GROW1 + A1], -1e30)
                nc.sync.dma_start(reg1[0:1, :], s1[:])

            # -------- chunked gather + parity select into Hs ---------------
            def gather_select(gp, Hs, idx_off, slots, grow, aw, regv):
                """gather pair rows, select parity into Hs[:, :slots//128, :aw].

                select = GE + (GO - GE) * msk  (one broadcast op instead of
                two; odd-parity rows pick up one bf16 rounding)."""
                if NOGATHER:
                    nc.vector.memset(Hs[:, :slots // 128, :], 0.0)
                    return
                GRP = GPC * GCAP
                gi = 0
                for g0 in range(0, slots, GRP):
                    gsl = min(GRP, slots - g0)
                    Gp = gp.tile([128, GRP // 128, 2 * grow], BF16, tag="Gp")
                    for c0 in range(g0, g0 + gsl, GCAP):
                        csl = min(GCAP, g0 + gsl - c0)
                        nc.gpsimd.dma_gather(
                            out_ap=Gp[:, (c0 - g0) // 128:
                                      (c0 - g0 + csl) // 128, :],
                            in_ap=regv,
                            idxs_ap=idxP_t[:, (idx_off + c0) // 16:
                                           (idx_off + c0 + csl) // 16],
                            num_idxs=csl, num_idxs_reg=nreg(csl),
                            elem_size=2 * grow, elem_step=2 * grow,
                            queue_num=gi % NQ)
                        gi += 1
                    nt = gsl // 128
                    cl = slice((idx_off + g0) // 128, (idx_off + g0 + gsl) // 128)
                    co = slice(g0 // 128, (g0 + gsl) // 128)
                    GE = Gp[:, :nt, :aw]
                    GO = Gp[:, :nt, grow:grow + aw]
                    nc.vector.tensor_tensor(GO, GO, GE, AL.subtract)
                    nc.vector.tensor_tensor(
                        GO, GO,
                        msk_t[:, cl].unsqueeze(2).broadcast_to((128, nt, aw)),
                        AL.mult)
                    nc.vector.tensor_tensor(Hs[:, co, :], GE, GO, AL.add)

            def tree_reduce(wk, Hv, nw_b, K, aw, ttag):
                """sum Hv[:, :, k, :aw] over k -> fp32 [128, nw_b, aw]."""
                m = K // 2
                T = wk.tile([128, nw_b, max((K + 1) // 2, 1), aw], F32,
                            tag=ttag)
                wcur = K - m
                if m > 0:
                    nc.vector.tensor_tensor(T[:, :, :m, :], Hv[:, :, :m, :aw],
                                            Hv[:, :, K - m:K, :aw], AL.add)
                if K % 2 == 1:
                    nc.vector.tensor_copy(T[:, :, m:m + 1, :],
                                          Hv[:, :, m:m + 1, :aw])
                while wcur > 1:
                    m2 = wcur // 2
                    nc.vector.tensor_tensor(
                        T[:, :, :m2, :], T[:, :, :m2, :],
                        T[:, :, wcur - m2:wcur, :], AL.add)
                    wcur -= m2
                return T

            # ---------------- layer 0 ----------------
            SPL1 = SPLC
            HA1, HB1 = SPL1 // 2, (OWNP - SPL1) // 2

            def kick1(lo, hi):
                o0_, o1_ = ((1, 1 + C * HA1) if lo == 0 else
                            (1 + C * HA1, 1 + C * (HA1 + HB1)))
                if NOCOLL:
                    nc.sync.dma_start(
                        reg1[o0_:o0_ + (hi - lo) // 2, :],
                        tblg1[lo:hi, :].rearrange("(v t) e -> v (t e)", t=2))
                else:
                    nc.gpsimd.collective_compute(
                        "AllGather", AL.bypass, replica_groups=rg,
                        ins=[tblg1[lo:hi, :].opt()],
                        outs=[reg1[o0_:o1_, :].opt()])

            trig = next(i for i, (w0, nw_b, K) in enumerate(batches)
                        if (w0 + nw_b) * 128 >= SPL1)
            idx_off = 0
            with tc.tile_pool(name="g0", bufs=2) as gp, \
                 tc.tile_pool(name="h0", bufs=2) as hp, \
                 tc.tile_pool(name="tp0", bufs=1) as tp, \
                 tc.tile_pool(name="t1ps", bufs=2, space="PSUM") as tps1, \
                 tc.tile_pool(name="wk0", bufs=2) as wk:
                for bi, (w0, nw_b, K) in enumerate(batches):
                    slots = nw_b * K * 128
                    Hs = hp.tile([128, MAX_TILES, AH], BF16, tag="Hs")
                    gather_select(gp, Hs, idx_off, slots, GROW0, AH,
                                  reg0[:, :])
                    idx_off += slots
                    Hv = Hs[:, :nw_b * K, :].rearrange(
                        "p (w k) e -> p w k e", w=nw_b)
                    s = wk.tile([128, nw_b, K, H], F32, tag="s")
                    nc.vector.tensor_tensor(
                        s[:], Hv[:, :, :, F_IN:AH],
                        ad0[:, w0:w0 + nw_b, :].unsqueeze(2)
                            .broadcast_to((128, nw_b, K, H)), AL.add)
                    nc.vector.scalar_tensor_tensor(s[:], s[:], NEG_SLOPE,
                                                   s[:], AL.mult, AL.max)
                    Ex = wk.tile([128, nw_b, K, H], BF16, tag="E")
                    nc.scalar.activation(Ex[:], s[:], ACT.Exp)
                    nc.scalar.activation(Hv[:, :, :, F_IN:AH], s[:], ACT.Exp)
                    Hm = Hs[:, :nw_b * K, :]
                    nc.vector.tensor_tensor(
                        Hm[:, :, :F_IN].rearrange("p m (h d) -> p m h d", h=H),
                        Hm[:, :, :F_IN].rearrange("p m (h d) -> p m h d", h=H),
                        Ex[:].rearrange("p w k h -> p (w k) h").unsqueeze(3)
                            .broadcast_to((128, nw_b * K, H, D)), AL.mult)
                    T = tree_reduce(tp, Hv, nw_b, K, AH, "T")
                    U = T[:, :, 0, :]
                    nc.vector.tensor_scalar(U[:, :, F_IN:], U[:, :, F_IN:],
                                            1e-16, None, AL.add)
                    rcp = wk.tile([128, nw_b, H], F32, tag="rcp")
                    nc.vector.reciprocal(rcp[:], U[:, :, F_IN:])
                    o0 = wk.tile([128, nw_b, F_IN], F32, tag="o0")
                    nc.vector.tensor_tensor(
                        o0[:].rearrange("p w (h d) -> p w h d", h=H),
                        U[:, :, :F_IN].rearrange("p w (h d) -> p w h d", h=H),
                        rcp[:].unsqueeze(3).broadcast_to((128, nw_b, H, D)),
                        AL.mult)
                    nc.vector.tensor_tensor(o0[:], o0[:],
                                            res0L[:, w0:w0 + nw_b, :], AL.add)
                    # ELU(x) = max(x, exp(min(x, 0)) - 1)
                    mn = wk.tile([128, nw_b, F_IN], F32, tag="mn")
                    nc.vector.tensor_scalar(mn[:], o0[:], 0.0, None, AL.min)
                    nc.scalar.activation(mn[:], mn[:], ACT.Exp)
                    h1 = wk.tile([128, nw_b, F_IN], BF16, tag="h1")
                    nc.vector.scalar_tensor_tensor(h1[:], mn[:], -1.0, o0[:],
                                                   AL.add, AL.max)
                    for wl in range(nw_b):
                        for k in range(KCH):
                            nc.sync.dma_start(
                                h1T[:, k, (w0 + wl) * 128:(w0 + wl + 1) * 128],
                                h1[:, wl, k * 128:(k + 1) * 128],
                                transpose=True)
                        wv = w0 + wl
                        ps1 = tps1.tile([128, C1], F32, tag="t1")
                        for k in range(KCH):
                            nc.tensor.matmul(
                                ps1[:], h1T[:, k, wv * 128:(wv + 1) * 128],
                                wcat1_t[:, k, :], start=(k == 0),
                                stop=(k == KCH - 1 and not BIAS1))
                        if BIAS1:
                            nc.tensor.matmul(ps1[:], ones_t[:], biasr1_t[:],
                                             start=False, stop=True)
                        st1 = wk.tile([128, GROW1], BF16, tag="st1")
                        nc.scalar.copy(st1[:, :A1], ps1[:, :A1])
                        nc.scalar.copy(ad1[:, wv, :], ps1[:, A1:C1])
                        nc.sync.dma_start(tblg1[wv * 128:(wv + 1) * 128, :],
                                          st1[:])
                    if bi == trig:
                        kick1(0, SPL1)

            kick1(SPL1, OWNP)

            # ---------------- layer 1 ----------------
            idx_off = 0
            with tc.tile_pool(name="g1", bufs=2) as gp, \
                 tc.tile_pool(name="h1p", bufs=2) as hp, \
                 tc.tile_pool(name="tp1", bufs=1) as tp, \
                 tc.tile_pool(name="wk1", bufs=2) as wk:
                for (w0, nw_b, K) in batches:
                    slots = nw_b * K * 128
                    Hs = hp.tile([128, MAX_TILES, A1], BF16, tag="Hs1")
                    gather_select(gp, Hs, idx_off, slots, GROW1, A1,
                                  reg1[:, :])
                    idx_off += slots
                    Hv = Hs[:, :nw_b * K, :].rearrange(
                        "p (w k) e -> p w k e", w=nw_b)
                    s = wk.tile([128, nw_b, K, 1], F32, tag="s")
                    nc.vector.tensor_tensor(
                        s[:], Hv[:, :, :, OUT:A1],
                        ad1[:, w0:w0 + nw_b, 0:1].unsqueeze(2)
                            .broadcast_to((128, nw_b, K, 1)), AL.add)
                    nc.vector.scalar_tensor_tensor(s[:], s[:], NEG_SLOPE,
                                                   s[:], AL.mult, AL.max)
                    Ex = wk.tile([128, nw_b, K, 1], BF16, tag="E")
                    nc.scalar.activation(Ex[:], s[:], ACT.Exp)
                    nc.scalar.activation(Hv[:, :, :, OUT:A1], s[:], ACT.Exp)
                    Hm = Hs[:, :nw_b * K, :]
                    nc.vector.tensor_tensor(
                        Hm[:, :, :OUT], Hm[:, :, :OUT],
                        Ex[:].rearrange("p w k h -> p (w k) h")
                            .broadcast_to((128, nw_b * K, OUT)), AL.mult)
                    T = tree_reduce(tp, Hv, nw_b, K, A1, "T1")
                    U = T[:, :, 0, :]
                    nc.vector.tensor_scalar(U[:, :, OUT:], U[:, :, OUT:],
                                            1e-16, None, AL.add)
                    rcp = wk.tile([128, nw_b, 1], F32, tag="rcp")
                    nc.vector.reciprocal(rcp[:], U[:, :, OUT:])
                    o = wk.tile([128, nw_b, OUT], F32, tag="o")
                    nc.vector.tensor_tensor(
                        o[:], U[:, :, :OUT],
                        rcp[:].broadcast_to((128, nw_b, OUT)), AL.mult)
                    nc.vector.tensor_tensor(o[:], o[:],
                                            ad1[:, w0:w0 + nw_b, 1:],
                                            AL.add)
                    # log_softmax (no shift needed: logits are O(10))
                    ev = wk.tile([128, nw_b, OUT], F32, tag="ev")
                    nc.scalar.activation(ev[:], o[:], ACT.Exp)
                    sv = wk.tile([128, nw_b, 1], F32, tag="sv")
                    nc.vector.reduce_sum(sv[:], ev[:], axis=mybir.AxisListType.X)
                    nc.scalar.activation(sv[:], sv[:], ACT.Ln)
                    nc.vector.tensor_tensor(
                        o[:], o[:], sv[:].broadcast_to((128, nw_b, OUT)),
                        AL.subtract)
                    nc.sync.dma_start(
                        out_own[w0 * 128:(w0 + nw_b) * 128, :].rearrange(
                            "(w p) e -> p w e", p=128), o[:])

    nc.compile()
    return nc


# ----------------------------------------------------------------- entrypoint

def kernel(x, edge_index, W0, a_src0, a_dst0, b0, Wr0, br0,
           W1, a_src1, a_dst1, b1, Wr1, br1):
    """Full-input GAT kernel: shards across 8 NeuronCores internally."""
    x = np.asarray(x)
    edge_index = np.asarray(edge_index)
    N, F_in = x.shape
    E = edge_index.shape[1]
    H, D = np.asarray(a_src0).shape
    OUT = np.asarray(a_src1).shape[1]
    cfg = make_cfg(N, E, F_in, H, D, OUT, _CORES)
    weights = dict(
        W0=np.asarray(W0, np.float32), a_src0=np.asarray(a_src0, np.float32),
        a_dst0=np.asarray(a_dst0, np.float32), b0=np.asarray(b0, np.float32),
        Wr0=np.asarray(Wr0, np.float32), br0=np.asarray(br0, np.float32),
        W1=np.asarray(W1, np.float32), a_src1=np.asarray(a_src1, np.float32),
        a_dst1=np.asarray(a_dst1, np.float32), b1=np.asarray(b1, np.float32),
        Wr1=np.asarray(Wr1, np.float32), br1=np.asarray(br1, np.float32))
    in_maps, meta = prepare(x.astype(np.float32), edge_index, weights, cfg)
    nc = build(cfg, meta)
    from concourse.bass_utils import run_bass_kernel_spmd
    res = run_bass_kernel_spmd(nc, in_maps, list(range(_CORES)))
    core_of, pos_of = meta["core_of"], meta["pos_of"]
    per_core = [np.asarray(res.results[c]["out_own"], np.float32)
                for c in range(_CORES)]
    stacked = np.stack(per_core)                       # [C, OWNP, OUT]
    out = stacked[core_of, pos_of]                     # [N, OUT]
    return out


# revision 6
# speedup vs baseline: 1.0408x; 1.0408x over previous
import sys
for _p in ("/opt/trn_rl_repo",):
    if _p not in sys.path:
        sys.path.insert(0, _p)
"""GAT 2-layer kernel for TRN2, 8-core dst-sharded — v10 "quarter collectives".

Gather cost on HW is ~8 ns per descriptor regardless of payload size, so
v3 issues ONE descriptor per edge (vs v2's two parity passes): the int16
index addresses a PAIR of adjacent table rows (25k pairs < 32767), the
descriptor fetches both rows (1280 B), and a per-slot 0/1 mask selects
the right parity on the Vector engine (2 multiplies + 1 add, bf16-exact).

Other changes vs v2:
  * residual/table matmuls emit node-major rows directly (lhs=x-window),
    eliminating the per-batch DMA transposes that stalled the Sync queue
  * the K-neighbor reduction is a log-tree of contiguous adds into an
    fp32 temp (the strided DVE reduce ran at ~0.5 elem/ns)
  * gathers are chunk-pipelined into a small staging tile and selected
    immediately, so batch i+1's gathers overlap batch i's vector work
"""

import numpy as np
import ml_dtypes
from contextlib import ExitStack

import concourse.bass as bass
import concourse.bacc as bacc
import concourse.mybir as mybir
import concourse.tile as tile

dt = mybir.dt
F32 = dt.float32
BF16 = dt.bfloat16
AL = mybir.AluOpType
ACT = mybir.ActivationFunctionType
BF = ml_dtypes.bfloat16

NEG_SLOPE = 0.2
_CORES = 8
WIN = 128
GCAP = 896
GPC = 2        # gather chunks per select group
MAX_TILES = 44
NQ = 4          # swdge queues for gather chunks


def wrap_idx(iv):
    """[n*16] int -> [128, n] int16 gather layout (t -> (t%16, t//16))."""
    iv = np.asarray(iv, np.int64)
    assert len(iv) % 16 == 0
    cols = len(iv) // 16
    a = iv.reshape(cols, 16).T.astype(np.int16)
    return np.tile(a, (8, 1))


def slot_grid(v, fill):
    """[TOTSLOT] values -> [128, TOTSLOT//128] (slot s -> [s%128, s//128])."""
    assert len(v) % 128 == 0
    return np.ascontiguousarray(np.asarray(v).reshape(-1, 128).T.astype(fill))


def make_cfg(N, E, F_in, H, D, OUT, cores):
    own = (N + cores - 1) // cores
    nw = (own + WIN - 1) // WIN
    ownp = nw * WIN
    assert ownp % 2 == 0
    cfg = dict(
        N=N, E=E, F_IN=F_in, H=H, D=D, OUT=OUT, CORES=cores, OWN=own,
        NW=nw, OWNP=ownp,
        GROW0=320, GROW1=128,          # gathered row elems per node
        C0=F_in + 2 * H,               # table0 matmul cols (h | al_s | al_d)
        C1=OUT + 2 + OUT,              # table1 cols (hW1 | als | ald | res)
        PROWS=1 + cores * ownp // 2,   # pair rows (row 0 = PAD sentinel)
        SEGS=tuple(min((nw * (i + 1) // 4 + 1) * WIN, ownp)
                   for i in range(4)),  # row splits for quarter collectives
    )
    assert cfg["PROWS"] - 1 <= 32767
    return cfg


def prepare(x, edge_index, weights, cfg):
    N, E, C, OWN, OWNP, NW = (cfg[k] for k in
                              ("N", "E", "CORES", "OWN", "OWNP", "NW"))
    H, D, F_IN, OUT = cfg["H"], cfg["D"], cfg["F_IN"], cfg["OUT"]

    src = np.concatenate([np.asarray(edge_index[0]),
                          np.arange(N)]).astype(np.int64)
    dst = np.concatenate([np.asarray(edge_index[1]),
                          np.arange(N)]).astype(np.int64)

    # degree-balanced relabeling: node order[i] -> (core i%C, pos i//C)
    deg = np.bincount(dst, minlength=N)
    order = np.argsort(-deg, kind="stable")
    core_of = np.empty(N, np.int64)
    pos_of = np.empty(N, np.int64)
    core_of[order] = np.arange(N) % C
    pos_of[order] = np.arange(N) // C

    # per-window max lane count (shared across cores)
    cnt = np.zeros((C, OWNP), np.int64)
    np.add.at(cnt, (core_of[dst], pos_of[dst]), 1)
    Kw = cnt.reshape(C, NW, WIN).max(axis=(0, 2))

    # batches of windows with uniform K, capped at MAX_TILES j-tiles
    batches = []
    w = 0
    while w < NW:
        K = max(int(Kw[w]), 1)
        assert K <= MAX_TILES, f"window degree {K} exceeds MAX_TILES"
        nw_b = 1
        while w + nw_b < NW:
            K2 = max(K, int(Kw[w + nw_b]))
            if (nw_b + 1) * K2 > MAX_TILES:
                break
            K = K2
            nw_b += 1
        batches.append((w, nw_b, K))
        w += nw_b

    slot0_w = np.zeros(NW, np.int64)
    base = 0
    for (w0, nw_b, K) in batches:
        for wl in range(nw_b):
            slot0_w[w0 + wl] = base + wl * K * WIN
        base += nw_b * K * WIN
    totslot = base
    assert totslot % 128 == 0

    # per-edge slot assignment: j = rank within (core,pos) group
    ec, ep = core_of[dst], pos_of[dst]
    order_e = np.lexsort((src, ep, ec))
    src_s, ec_s, ep_s = src[order_e], ec[order_e], ep[order_e]
    grp = ec_s * OWNP + ep_s
    starts = np.searchsorted(grp, np.arange(C * OWNP))
    j_of = np.arange(len(grp)) - starts[grp]
    wd = ep_s // WIN
    lane = ep_s % WIN
    slot = slot0_w[wd] + j_of * WIN + lane
    # pair layout: for each row segment (quarter), all cores' segment rows
    # sit contiguously — so each quarter all-gather writes contiguously
    SEGS = cfg["SEGS"]
    bounds = (0,) + SEGS
    sc, sp = core_of[src_s], pos_of[src_s]
    pair = np.zeros(len(sp), np.int64)
    segbase = 1
    for si in range(4):
        lo, hi = bounds[si], bounds[si + 1]
        hs = (hi - lo) // 2
        m_ = (sp >= lo) & (sp < hi)
        pair[m_] = segbase + sc[m_] * hs + (sp[m_] - lo) // 2
        segbase += C * hs
    parity = sp % 2

    in_maps = []
    for c in range(C):
        m = ec_s == c
        pr = np.zeros(totslot, np.int64)           # default: sentinel pair
        pa = np.zeros(totslot, np.int64)
        pr[slot[m]] = pair[m]
        pa[slot[m]] = parity[m]
        nodes = np.full(OWNP, -1, np.int64)
        mine = core_of == c
        nodes[pos_of[mine]] = np.where(mine)[0]
        xT = np.zeros((F_IN, OWNP), BF)
        valid = nodes >= 0
        xT[:, valid] = np.asarray(x, np.float32)[nodes[valid]].T.astype(BF)
        in_maps.append(dict(
            xT=xT,
            idxP=wrap_idx(pr),
            msk=slot_grid(pa, BF),
            mski=slot_grid(1 - pa, BF),
        ))

    # shared constants
    W0 = weights["W0"]; a_s0 = weights["a_src0"]; a_d0 = weights["a_dst0"]
    Wr0 = weights["Wr0"]; W1 = weights["W1"]; a_s1 = weights["a_src1"]
    a_d1 = weights["a_dst1"]; Wr1 = weights["Wr1"]
    blk0s = np.zeros((H * D, H), np.float32)
    blk0d = np.zeros((H * D, H), np.float32)
    for k in range(H):
        blk0s[k * D:(k + 1) * D, k] = a_s0[k]
        blk0d[k * D:(k + 1) * D, k] = a_d0[k]
    wcat0 = np.concatenate([W0, W0 @ blk0s, W0 @ blk0d], axis=1).astype(BF)
    wcat1 = np.concatenate([W1, W1 @ a_s1.reshape(-1, 1),
                            W1 @ a_d1.reshape(-1, 1), Wr1], axis=1).astype(BF)
    biasr0 = (weights["b0"] + weights["br0"]).astype(BF).reshape(1, -1)
    biasr1 = np.zeros((1, cfg["C1"]), BF)
    biasr1[0, OUT + 2:] = (weights["b1"] + weights["br1"]).astype(BF)
    consts = dict(
        wcat0=wcat0, wr0=np.asarray(Wr0, np.float32).astype(BF),
        wcat1=wcat1, biasr0=biasr0, biasr1=biasr1,
    )
    for m_ in in_maps:
        m_.update(consts)
    meta = dict(batches=batches, totslot=totslot,
                core_of=core_of, pos_of=pos_of,
                bias0_nz=bool(np.any(np.asarray(biasr0, np.float32))),
                bias1_nz=bool(np.any(np.asarray(biasr1, np.float32))))
    return in_maps, meta


def build(cfg, meta, repeat=1, abl=()):
    NOCOLL = "nocoll" in abl
    NOGATHER = "nogather" in abl
    N, C, OWN, OWNP, NW = (cfg[k] for k in
                           ("N", "CORES", "OWN", "OWNP", "NW"))
    F_IN, H, D, OUT = cfg["F_IN"], cfg["H"], cfg["D"], cfg["OUT"]
    GROW0, GROW1, C0, C1 = cfg["GROW0"], cfg["GROW1"], cfg["C0"], cfg["C1"]
    PROWS = cfg["PROWS"]
    SEGSC = cfg["SEGS"]
    SEGB = (0,) + SEGSC
    SEGOUT = [1]                       # reg row base of each segment
    for _si in range(4):
        SEGOUT.append(SEGOUT[-1] + C * (SEGB[_si + 1] - SEGB[_si]) // 2)
    batches = meta["batches"]
    TOTSLOT = meta["totslot"]
    BIAS0 = meta.get("bias0_nz", True)
    BIAS1 = meta.get("bias1_nz", True)
    KCH = (F_IN + 127) // 128   # 2
    AH = F_IN + H               # 264: selected row [h | al_s]
    A1 = OUT + 1                # 65: selected row [hW1 | als]

    nc = bacc.Bacc("TRN2", target_bir_lowering=False, debug=False,
                   num_devices=C, num_swdge_queues=NQ)

    _nreg_cache = {}

    def nreg(v):
        if v not in _nreg_cache:
            _nreg_cache[v] = nc.gpsimd.to_reg(v)
        return _nreg_cache[v]

    xT_in = nc.dram_tensor("xT", [F_IN, OWNP], BF16, kind="ExternalInput")
    idxP_in = nc.dram_tensor("idxP", [128, TOTSLOT // 16], dt.int16,
                             kind="ExternalInput")
    msk_in = nc.dram_tensor("msk", [128, TOTSLOT // 128], BF16,
                            kind="ExternalInput")
    mski_in = nc.dram_tensor("mski", [128, TOTSLOT // 128], BF16,
                             kind="ExternalInput")
    wcat0_in = nc.dram_tensor("wcat0", [F_IN, C0], BF16, kind="ExternalInput")
    wr0_in = nc.dram_tensor("wr0", [F_IN, F_IN], BF16, kind="ExternalInput")
    wcat1_in = nc.dram_tensor("wcat1", [F_IN, C1], BF16, kind="ExternalInput")
    biasr0_in = nc.dram_tensor("biasr0", [1, F_IN], BF16, kind="ExternalInput")
    biasr1_in = nc.dram_tensor("biasr1", [1, C1], BF16, kind="ExternalInput")
    out_own = nc.dram_tensor("out_own", [OWNP, OUT], F32, kind="ExternalOutput")

    tblg0 = nc.dram_tensor("tblg0", [OWNP, GROW0], BF16)
    tblg1 = nc.dram_tensor("tblg1", [OWNP, GROW1], BF16)
    reg0 = nc.dram_tensor("reg0", [PROWS, 2 * GROW0], BF16, addr_space="Shared")
    reg1 = nc.dram_tensor("reg1", [PROWS, 2 * GROW1], BF16, addr_space="Shared")
    rg = [list(range(C))]

    with tile.TileContext(nc) as tc, ExitStack() as ctx:
        const = ctx.enter_context(tc.tile_pool(name="const", bufs=1))
        wcat0_t = const.tile([128, KCH, C0], BF16)
        nc.sync.dma_start(wcat0_t[:], wcat0_in[:, :].rearrange(
            "(k p) c -> p k c", p=128))
        wr0_t = const.tile([128, KCH, F_IN], BF16)
        nc.sync.dma_start(wr0_t[:], wr0_in[:, :].rearrange(
            "(k p) c -> p k c", p=128))
        wcat1_t = const.tile([128, KCH, C1], BF16)
        nc.sync.dma_start(wcat1_t[:], wcat1_in[:, :].rearrange(
            "(k p) c -> p k c", p=128))
        biasr0_t = const.tile([1, F_IN], BF16)
        nc.sync.dma_start(biasr0_t[:], biasr0_in[:])
        biasr1_t = const.tile([1, C1], BF16)
        nc.sync.dma_start(biasr1_t[:], biasr1_in[:])
        ones_t = const.tile([1, 128], BF16)
        nc.vector.memset(ones_t[:], 1.0)
        idxP_t = const.tile([128, TOTSLOT // 16], dt.int16)
        nc.sync.dma_start(idxP_t[:], idxP_in[:])
        msk_t = const.tile([128, TOTSLOT // 128], BF16)
        nc.sync.dma_start(msk_t[:], msk_in[:])
        mski_t = const.tile([128, TOTSLOT // 128], BF16)
        nc.sync.dma_start(mski_t[:], mski_in[:])

        persist = ctx.enter_context(tc.tile_pool(name="persist", bufs=1))
        h1T = persist.tile([128, KCH, OWNP], BF16)
        res0L = persist.tile([128, NW, F_IN], BF16)
        ad0 = persist.tile([128, NW, H], BF16)
        ad1 = persist.tile([128, NW, 1 + OUT], BF16)

        for _rep in range(repeat):
            # ---------------- setup: table0 rows, res0, sentinels ----------
            with tc.tile_pool(name="xp", bufs=1) as xp, \
                 tc.tile_pool(name="sps", bufs=2, space="PSUM") as sps, \
                 tc.tile_pool(name="ssb", bufs=2) as ssb:
                xT_t = xp.tile([128, KCH, OWNP], BF16)
                nc.sync.dma_start(xT_t[:], xT_in[:, :].rearrange(
                    "(k p) n -> p k n", p=128))
                # quarter all-gathers: each fires as soon as its window
                # segment's table rows are written, hiding behind the rest
                # of table0 + the res0 matmuls; only the last quarter's
                # latency is exposed
                def kick0(si):
                    lo, hi = SEGB[si], SEGB[si + 1]
                    if lo >= hi:
                        return
                    if NOCOLL:
                        nc.sync.dma_start(
                            reg0[SEGOUT[si]:SEGOUT[si] + (hi - lo) // 2, :],
                            tblg0[lo:hi, :].rearrange("(v t) e -> v (t e)", t=2))
                    else:
                        nc.gpsimd.collective_compute(
                            "AllGather", AL.bypass, replica_groups=rg,
                            ins=[tblg0[lo:hi, :].opt()],
                            outs=[reg0[SEGOUT[si]:SEGOUT[si + 1], :].opt()])

                for w in range(NW):
                    ps = sps.tile([128, C0], F32, tag="t0")
                    for k in range(KCH):
                        nc.tensor.matmul(ps[:], xT_t[:, k, w * 128:(w + 1) * 128],
                                         wcat0_t[:, k, :],
                                         start=(k == 0), stop=(k == KCH - 1))
                    st = ssb.tile([128, GROW0], BF16, tag="st0")
                    nc.scalar.copy(st[:, :AH], ps[:, :AH])
                    nc.scalar.copy(ad0[:, w, :], ps[:, AH:C0])
                    nc.sync.dma_start(tblg0[w * 128:(w + 1) * 128, :], st[:])
                    for si in range(4):
                        if (w + 1) * 128 == SEGSC[si]:
                            kick0(si)
                for w in range(NW):
                    ps = sps.tile([128, F_IN], F32, tag="r0")
                    for k in range(KCH):
                        nc.tensor.matmul(ps[:], xT_t[:, k, w * 128:(w + 1) * 128],
                                         wr0_t[:, k, :], start=(k == 0),
                                         stop=(k == KCH - 1 and not BIAS0))
                    if BIAS0:
                        nc.tensor.matmul(ps[:], ones_t[:], biasr0_t[:],
                                         start=False, stop=True)
                    nc.scalar.copy(res0L[:, w, :], ps[:])
                # PAD sentinel pair rows (al_s = -1e30 in both halves)
                s0 = ssb.tile([1, 2 * GROW0], BF16, tag="s0")
                nc.vector.memset(s0[:], 0.0)
                nc.vector.memset(s0[:, F_IN:AH], -1e30)
                nc.vector.memset(s0[:, GROW0 + F_IN:GROW0 + AH], -1e30)
                nc.sync.dma_start(reg0[0:1, :], s0[:])
                s1 = ssb.tile([1, 2 * GROW1], BF16, tag="s1")
                nc.vector.memset(s1[:], 0.0)
                nc.vector.memset(s1[:, OUT:A1], -1e30)
                nc.vector.memset(s1[:, GROW1 + OUT:GROW1 + A1], -1e30)
                nc.sync.dma_start(reg1[0:1, :], s1[:])

            # -------- chunked gather + parity select into Hs ---------------
            def gather_select(gp, Hs, idx_off, slots, grow, aw, regv):
                """gather pair rows, select parity into Hs[:, :slots//128, :aw].

                select = GE + (GO - GE) * msk  (one broadcast op instead of
                two; odd-parity rows pick up one bf16 rounding)."""
                if NOGATHER:
                    nc.vector.memset(Hs[:, :slots // 128, :], 0.0)
                    return
                GRP = GPC * GCAP
                gi = 0
                for g0 in range(0, slots, GRP):
                    gsl = min(GRP, slots - g0)
                    Gp = gp.tile([128, GRP // 128, 2 * grow], BF16, tag="Gp")
                    for c0 in range(g0, g0 + gsl, GCAP):
                        csl = min(GCAP, g0 + gsl - c0)
                        nc.gpsimd.dma_gather(
                            out_ap=Gp[:, (c0 - g0) // 128:
                                      (c0 - g0 + csl) // 128, :],
                            in_ap=regv,
                            idxs_ap=idxP_t[:, (idx_off + c0) // 16:
                                           (idx_off + c0 + csl) // 16],
                            num_idxs=csl, num_idxs_reg=nreg(csl),
                            elem_size=2 * grow, elem_step=2 * grow,
                            queue_num=gi % NQ)
                        gi += 1
                    nt = gsl // 128
                    cl = slice((idx_off + g0) // 128, (idx_off + g0 + gsl) // 128)
                    co = slice(g0 // 128, (g0 + gsl) // 128)
                    GE = Gp[:, :nt, :aw]
                    GO = Gp[:, :nt, grow:grow + aw]
                    nc.vector.tensor_tensor(GO, GO, GE, AL.subtract)
                    nc.vector.tensor_tensor(
                        GO, GO,
                        msk_t[:, cl].unsqueeze(2).broadcast_to((128, nt, aw)),
                        AL.mult)
                    nc.vector.tensor_tensor(Hs[:, co, :], GE, GO, AL.add)

            def tree_reduce(wk, Hv, nw_b, K, aw, ttag):
                """sum Hv[:, :, k, :aw] over k -> fp32 [128, nw_b, aw]."""
                m = K // 2
                T = wk.tile([128, nw_b, max((K + 1) // 2, 1), aw], F32,
                            tag=ttag)
                wcur = K - m
                if m > 0:
                    nc.vector.tensor_tensor(T[:, :, :m, :], Hv[:, :, :m, :aw],
                                            Hv[:, :, K - m:K, :aw], AL.add)
                if K % 2 == 1:
                    nc.vector.tensor_copy(T[:, :, m:m + 1, :],
                                          Hv[:, :, m:m + 1, :aw])
                while wcur > 1:
                    m2 = wcur // 2
                    nc.vector.tensor_tensor(
                        T[:, :, :m2, :], T[:, :, :m2, :],
                        T[:, :, wcur - m2:wcur, :], AL.add)
                    wcur -= m2
                return T

            # ---------------- layer 0 ----------------
            def kick1(si):
                lo, hi = SEGB[si], SEGB[si + 1]
                if lo >= hi:
                    return
                if NOCOLL:
                    nc.sync.dma_start(
                        reg1[SEGOUT[si]:SEGOUT[si] + (hi - lo) // 2, :],
                        tblg1[lo:hi, :].rearrange("(v t) e -> v (t e)", t=2))
                else:
                    nc.gpsimd.collective_compute(
                        "AllGather", AL.bypass, replica_groups=rg,
                        ins=[tblg1[lo:hi, :].opt()],
                        outs=[reg1[SEGOUT[si]:SEGOUT[si + 1], :].opt()])

            # trigger batch for each quarter: first batch covering its rows
            trigs = {}
            for si in range(3):
                bi_ = next(i for i, (w0, nw_b, K) in enumerate(batches)
                           if (w0 + nw_b) * 128 >= SEGSC[si])
                trigs.setdefault(bi_, []).append(si)
            idx_off = 0
            with tc.tile_pool(name="g0", bufs=2) as gp, \
                 tc.tile_pool(name="h0", bufs=2) as hp, \
                 tc.tile_pool(name="tp0", bufs=1) as tp, \
                 tc.tile_pool(name="t1ps", bufs=2, space="PSUM") as tps1, \
                 tc.tile_pool(name="wk0", bufs=2) as wk:
                for bi, (w0, nw_b, K) in enumerate(batches):
                    slots = nw_b * K * 128
                    Hs = hp.tile([128, MAX_TILES, AH], BF16, tag="Hs")
                    gather_select(gp, Hs, idx_off, slots, GROW0, AH,
                                  reg0[:, :])
                    idx_off += slots
                    Hv = Hs[:, :nw_b * K, :].rearrange(
                        "p (w k) e -> p w k e", w=nw_b)
                    s = wk.tile([128, nw_b, K, H], F32, tag="s")
                    nc.vector.tensor_tensor(
                        s[:], Hv[:, :, :, F_IN:AH],
                        ad0[:, w0:w0 + nw_b, :].unsqueeze(2)
                            .broadcast_to((128, nw_b, K, H)), AL.add)
                    nc.vector.scalar_tensor_tensor(s[:], s[:], NEG_SLOPE,
                                                   s[:], AL.mult, AL.max)
                    Ex = wk.tile([128, nw_b, K, H], BF16, tag="E")
                    nc.scalar.activation(Ex[:], s[:], ACT.Exp)
                    nc.scalar.activation(Hv[:, :, :, F_IN:AH], s[:], ACT.Exp)
                    Hm = Hs[:, :nw_b * K, :]
                    nc.vector.tensor_tensor(
                        Hm[:, :, :F_IN].rearrange("p m (h d) -> p m h d", h=H),
                        Hm[:, :, :F_IN].rearrange("p m (h d) -> p m h d", h=H),
                        Ex[:].rearrange("p w k h -> p (w k) h").unsqueeze(3)
                            .broadcast_to((128, nw_b * K, H, D)), AL.mult)
                    T = tree_reduce(tp, Hv, nw_b, K, AH, "T")
                    U = T[:, :, 0, :]
                    nc.vector.tensor_scalar(U[:, :, F_IN:], U[:, :, F_IN:],
                                            1e-16, None, AL.add)
                    rcp = wk.tile([128, nw_b, H], F32, tag="rcp")
                    nc.vector.reciprocal(rcp[:], U[:, :, F_IN:])
                    o0 = wk.tile([128, nw_b, F_IN], F32, tag="o0")
                    nc.vector.tensor_tensor(
                        o0[:].rearrange("p w (h d) -> p w h d", h=H),
                        U[:, :, :F_IN].rearrange("p w (h d) -> p w h d", h=H),
                        rcp[:].unsqueeze(3).broadcast_to((128, nw_b, H, D)),
                        AL.mult)
                    nc.vector.tensor_tensor(o0[:], o0[:],
                                            res0L[:, w0:w0 + nw_b, :], AL.add)
                    # ELU(x) = max(x, exp(min(x, 0)) - 1)
                    mn = wk.tile([128, nw_b, F_IN], F32, tag="mn")
                    nc.vector.tensor_scalar(mn[:], o0[:], 0.0, None, AL.min)
                    nc.scalar.activation(mn[:], mn[:], ACT.Exp)
                    h1 = wk.tile([128, nw_b, F_IN], BF16, tag="h1")
                    nc.vector.scalar_tensor_tensor(h1[:], mn[:], -1.0, o0[:],
                                                   AL.add, AL.max)
                    for wl in range(nw_b):
                        for k in range(KCH):
                            nc.sync.dma_start(
                                h1T[:, k, (w0 + wl) * 128:(w0 + wl + 1) * 128],
                                h1[:, wl, k * 128:(k + 1) * 128],
                                transpose=True)
                        wv = w0 + wl
                        ps1 = tps1.tile([128, C1], F32, tag="t1")
                        for k in range(KCH):
                            nc.tensor.matmul(
                                ps1[:], h1T[:, k, wv * 128:(wv + 1) * 128],
                                wcat1_t[:, k, :], start=(k == 0),
                                stop=(k == KCH - 1 and not BIAS1))
                        if BIAS1:
                            nc.tensor.matmul(ps1[:], ones_t[:], biasr1_t[:],
                                             start=False, stop=True)
                        st1 = wk.tile([128, GROW1], BF16, tag="st1")
                        nc.scalar.copy(st1[:, :A1], ps1[:, :A1])
                        nc.scalar.copy(ad1[:, wv, :], ps1[:, A1:C1])
                        nc.sync.dma_start(tblg1[wv * 128:(wv + 1) * 128, :],
                                          st1[:])
                    for si in trigs.get(bi, ()):
                        kick1(si)

            kick1(3)

            # ---------------- layer 1 ----------------
            idx_off = 0
            with tc.tile_pool(name="g1", bufs=2) as gp, \
                 tc.tile_pool(name="h1p", bufs=2) as hp, \
                 tc.tile_pool(name="tp1", bufs=1) as tp, \
                 tc.tile_pool(name="wk1", bufs=2) as wk:
                for (w0, nw_b, K) in batches:
                    slots = nw_b * K * 128
                    Hs = hp.tile([128, MAX_TILES, A1], BF16, tag="Hs1")
                    gather_select(gp, Hs, idx_off, slots, GROW1, A1,
                                  reg1[:, :])
                    idx_off += slots
                    Hv = Hs[:, :nw_b * K, :].rearrange(
                        "p (w k) e -> p w k e", w=nw_b)
                    s = wk.tile([128, nw_b, K, 1], F32, tag="s")
                    nc.vector.tensor_tensor(
                        s[:], Hv[:, :, :, OUT:A1],
                        ad1[:, w0:w0 + nw_b, 0:1].unsqueeze(2)
                            .broadcast_to((128, nw_b, K, 1)), AL.add)
                    nc.vector.scalar_tensor_tensor(s[:], s[:], NEG_SLOPE,
                                                   s[:], AL.mult, AL.max)
                    Ex = wk.tile([128, nw_b, K, 1], BF16, tag="E")
                    nc.scalar.activation(Ex[:], s[:], ACT.Exp)
                    nc.scalar.activation(Hv[:, :, :, OUT:A1], s[:], ACT.Exp)
                    Hm = Hs[:, :nw_b * K, :]
                    nc.vector.tensor_tensor(
                        Hm[:, :, :OUT], Hm[:, :, :OUT],
                        Ex[:].rearrange("p w k h -> p (w k) h")
                            .broadcast_to((128, nw_b * K, OUT)), AL.mult)
                    T = tree_reduce(tp, Hv, nw_b, K, A1, "T1")
                    U = T[:, :, 0, :]
                    nc.vector.tensor_scalar(U[:, :, OUT:], U[:, :, OUT:],
                                            1e-16, None, AL.add)
                    rcp = wk.tile([128, nw_b, 1], F32, tag="rcp")
                    nc.vector.reciprocal(rcp[:], U[:, :, OUT:])
                    o = wk.tile([128, nw_b, OUT], F32, tag="o")
                    nc.vector.tensor_tensor(
                        o[:], U[:, :, :OUT],
                        rcp[:].broadcast_to((128, nw_b, OUT)), AL.mult)
                    nc.vector.tensor_tensor(o[:], o[:],
                                            ad1[:, w0:w0 + nw_b, 1:],
                                            AL.add)
                    # log_softmax (no shift needed: logits are O(10))
                    ev = wk.tile([128, nw_b, OUT], F32, tag="ev")
                    nc.scalar.activation(ev[:], o[:], ACT.Exp)
                    sv = wk.tile([128, nw_b, 1], F32, tag="sv")
                    nc.vector.reduce_sum(sv[:], ev[:], axis=mybir.AxisListType.X)
                    nc.scalar.activation(sv[:], sv[:], ACT.Ln)
                    nc.vector.tensor_tensor(
                        o[:], o[:], sv[:].broadcast_to((128, nw_b, OUT)),
                        AL.subtract)
                    nc.sync.dma_start(
                        out_own[w0 * 128:(w0 + nw_b) * 128, :].rearrange(
                            "(w p) e -> p w e", p=128), o[:])

    nc.compile()
    return nc


# ----------------------------------------------------------------- entrypoint

def kernel(x, edge_index, W0, a_src0, a_dst0, b0, Wr0, br0,
           W1, a_src1, a_dst1, b1, Wr1, br1):
    """Full-input GAT kernel: shards across 8 NeuronCores internally."""
    x = np.asarray(x)
    edge_index = np.asarray(edge_index)
    N, F_in = x.shape
    E = edge_index.shape[1]
    H, D = np.asarray(a_src0).shape
    OUT = np.asarray(a_src1).shape[1]
    cfg = make_cfg(N, E, F_in, H, D, OUT, _CORES)
    weights = dict(
        W0=np.asarray(W0, np.float32), a_src0=np.asarray(a_src0, np.float32),
        a_dst0=np.asarray(a_dst0, np.float32), b0=np.asarray(b0, np.float32),
        Wr0=np.asarray(Wr0, np.float32), br0=np.asarray(br0, np.float32),
        W1=np.asarray(W1, np.float32), a_src1=np.asarray(a_src1, np.float32),
        a_dst1=np.asarray(a_dst1, np.float32), b1=np.asarray(b1, np.float32),
        Wr1=np.asarray(Wr1, np.float32), br1=np.asarray(br1, np.float32))
    in_maps, meta = prepare(x.astype(np.float32), edge_index, weights, cfg)
    nc = build(cfg, meta)
    from concourse.bass_utils import run_bass_kernel_spmd
    res = run_bass_kernel_spmd(nc, in_maps, list(range(_CORES)))
    core_of, pos_of = meta["core_of"], meta["pos_of"]
    per_core = [np.asarray(res.results[c]["out_own"], np.float32)
                for c in range(_CORES)]
    stacked = np.stack(per_core)                       # [C, OWNP, OUT]
    out = stacked[core_of, pos_of]                     # [N, OUT]
    return out
